# revision 1
# baseline (speedup 1.0000x reference)
"""Trainium2 Bass kernel for nn_CrossAttentionBlock (LN -> MHA -> out-proj -> residual).

Sharding: 8 cores = 2 batches x 4 head-groups (2 heads each). Each core:
  - computes LN stats of its batch's x ([c, seq] layout, stats via ones-matmul),
  - projects Q/K/V for its 2 heads (gamma/beta folded into weights on host),
  - flash-style attention: exp on ACT (bf16 probs), AV+sumexp via [1|V] matmul,
  - partial out-proj with its Wo rows.
Host sums the 4 partials per batch and adds bias + residual.
"""
import numpy as np

C = 512
SEQ = 2048
P = 128
NCH = 4          # c chunks of 128
DH = 64
HPC = 2          # heads per core
EPS = 1e-5

_CACHE = {}
_LAST_IN_MAPS = None


def _build():
    import concourse.bass as bass
    import concourse.tile as tile
    from concourse import bacc, mybir
    from concourse.masks import make_identity

    F32 = mybir.dt.float32
    F32R = mybir.dt.float32r
    BF16 = mybir.dt.bfloat16
    AF = mybir.ActivationFunctionType
    ALU = mybir.AluOpType

    nc = bacc.Bacc("TRN2", target_bir_lowering=False, debug=False,
                   enable_asserts=False, num_devices=8)

    xb_d = nc.dram_tensor("xb", [C, SEQ], F32, kind="ExternalInput").ap()
    aq_d = nc.dram_tensor("aq", [C, P], F32, kind="ExternalInput").ap()
    ak_d = nc.dram_tensor("ak", [C, P], F32, kind="ExternalInput").ap()
    av_d = nc.dram_tensor("av", [C, P], F32, kind="ExternalInput").ap()
    wo_d = nc.dram_tensor("wo", [P, C], F32, kind="ExternalInput").ap()
    uq_d = nc.dram_tensor("uq", [1, P], F32, kind="ExternalInput").ap()
    uk_d = nc.dram_tensor("uk", [1, P], F32, kind="ExternalInput").ap()
    uv_d = nc.dram_tensor("uv", [1, P], F32, kind="ExternalInput").ap()
    vq_d = nc.dram_tensor("vq", [P, 1], F32, kind="ExternalInput").ap()
    vk_d = nc.dram_tensor("vk", [P, 1], F32, kind="ExternalInput").ap()
    yp_d = nc.dram_tensor("yp", [C, SEQ], F32, kind="ExternalOutput").ap()

    with tile.TileContext(nc) as tc:
        with tc.tile_pool(name="sb", bufs=1) as sb, \
             tc.tile_pool(name="ep", bufs=4) as ep, \
             tc.tile_pool(name="pa", bufs=1, space="PSUM") as pa, \
             tc.tile_pool(name="pb", bufs=1, space="PSUM") as pb:

            # ---- constant / weight loads ----
            aw = {}
            for name, d in (("aq", aq_d), ("ak", ak_d), ("av", av_d)):
                t = sb.tile([P, NCH, P], F32R, tag=name)
                nc.sync.dma_start(
                    t[:], d.rearrange("(k p) m -> p k m", p=P).bitcast(F32R))
                aw[name] = t
            wo_t = sb.tile([P, C], F32R, tag="wo")
            nc.sync.dma_start(wo_t[:], wo_d[:, :].bitcast(F32R))
            uvec = {}
            for name, d in (("uq", uq_d), ("uk", uk_d), ("uv", uv_d)):
                t = sb.tile([1, P], F32R, tag=name)
                nc.sync.dma_start(t[:], d[:, :].bitcast(F32R))
                uvec[name] = t
            vq_t = sb.tile([P, 1], F32, tag="vq")
            nc.sync.dma_start(vq_t[:], vq_d[:, :])
            vk_t = sb.tile([P, 1], F32, tag="vk")
            nc.sync.dma_start(vk_t[:], vk_d[:, :])
            ones_f = sb.tile([P, 2], F32, tag="onesf")
            nc.vector.memset(ones_f[:], 1.0 / C)
            ones_t = sb.tile([P, 2], F32R, tag="ones")
            nc.vector.tensor_copy(ones_t[:], ones_f[:])
            eps_t = sb.tile([1, 1], F32, tag="eps")
            nc.vector.memset(eps_t[:], EPS)
            ident_f = sb.tile([P, P], F32, tag="idf")
            make_identity(nc, ident_f[:])
            ident = sb.tile([P, P], F32R, tag="id")
            nc.vector.tensor_copy(ident[:], ident_f[:])

            # ---- x load + square ----
            xt = []
            for k in range(NCH):
                t = sb.tile([P, SEQ], F32R, tag=f"x{k}")
                nc.sync.dma_start(t[:], xb_d[k * P:(k + 1) * P, :].bitcast(F32R))
                xt.append(t)
            xsq = []
            for k in range(NCH):
                t = sb.tile([P, SEQ], F32R, tag=f"q{k}")
                nc.vector.tensor_tensor(t[:], xt[k][:], xt[k][:], ALU.mult)
                xsq.append(t)

            # ---- LN stats: s1 then s2 through the same psum tile ----
            st_ps = pb.tile([2, NCH, 512], F32, tag="b")
            for nb in range(NCH):
                for k in range(NCH):
                    nc.tensor.matmul(st_ps[0:2, nb, :], ones_t[:],
                                     xt[k][:, nb * 512:(nb + 1) * 512],
                                     start=(k == 0), stop=(k == 3))
            mu_sb = sb.tile([1, SEQ], F32, tag="musb")
            nc.vector.tensor_copy(mu_sb[:], st_ps[0:1, :, :])
            st2_ps = pb.tile([2, NCH, 512], F32, tag="b")
            for nb in range(NCH):
                for k in range(NCH):
                    nc.tensor.matmul(st2_ps[0:2, nb, :], ones_t[:],
                                     xsq[k][:, nb * 512:(nb + 1) * 512],
                                     start=(k == 0), stop=(k == 3))
            mu = mu_sb[0:1, :]
            e2 = st2_ps[0:1, :, :]
            musq = sb.tile([1, SEQ], F32, tag="musq")
            nc.vector.tensor_tensor(musq[:], mu, mu, ALU.mult)
            varr = sb.tile([1, SEQ], F32, tag="varr")
            nc.vector.tensor_tensor(varr[:], e2, musq[:], ALU.subtract)
            lnv = sb.tile([1, SEQ], F32, tag="lnv")
            nc.scalar.activation(lnv[:], varr[:], AF.Ln, bias=eps_t[0:1, :], scale=1.0)
            rs_row = sb.tile([1, SEQ], F32, tag="rsr")
            nc.scalar.activation(rs_row[:], lnv[:], AF.Exp, bias=0.0, scale=-0.5)
            m2_row = sb.tile([1, SEQ], F32R, tag="m2r")
            nc.vector.tensor_tensor(m2_row[:], mu, rs_row[:], ALU.mult)
            rs_b = sb.tile([P, SEQ], F32, tag="rsb")
            nc.gpsimd.partition_broadcast(rs_b[:], rs_row[:], channels=P)

            # ---- z = x * rs (per-token scale; mean-shift folded via rank-1) ----
            z = []
            for k in range(NCH):
                t = sb.tile([P, SEQ], F32R, tag=f"q{k}")
                nc.vector.tensor_tensor(t[:], xt[k][:], rs_b[:], ALU.mult)
                z.append(t)

            # ---- projections ----
            def project(w_t, u_t, out_sb, bias_t):
                for nb in range(4):
                    slot = pa.tile([P, 512], F32, tag=f"s{nb % 2}")
                    for k in range(NCH):
                        nc.tensor.matmul(slot[:], w_t[:, k, :],
                                         z[k][:, nb * 512:(nb + 1) * 512],
                                         start=(k == 0), stop=False)
                    nc.tensor.matmul(slot[:], u_t[:],
                                     m2_row[0:1, nb * 512:(nb + 1) * 512],
                                     start=False, stop=True)
                    if bias_t is not None:
                        nc.vector.tensor_scalar(
                            out=out_sb[:, nb * 512:(nb + 1) * 512], in0=slot[:],
                            scalar1=bias_t[:, 0:1], scalar2=None, op0=ALU.add)
                    else:
                        nc.vector.tensor_copy(
                            out_sb[:, nb * 512:(nb + 1) * 512], slot[:])

            qt_sb = sb.tile([P, SEQ], F32R, tag="qt")
            project(aw["aq"], uvec["uq"], qt_sb, vq_t)
            kt_sb = sb.tile([P, SEQ], F32R, tag="kt")
            project(aw["ak"], uvec["uk"], kt_sb, vk_t)
            vt_sb = sb.tile([P, SEQ], F32R, tag="vt")
            project(aw["av"], uvec["uv"], vt_sb, None)

            # ---- V -> [j, d] layout via PE transpose; bf16 [1|0*63|V64] per head ----
            v_sb = sb.tile([P, 16, 256], BF16, tag="vsb")
            nc.vector.memset(v_sb[:], 0.0)
            nc.vector.memset(
                v_sb[:].rearrange("p j (h c) -> p j h c", c=128)[:, :, :, 0:1], 1.0)
            for jb in range(16):
                tr = pa.tile([P, P], F32R, tag=f"s{jb % 2}")
                nc.tensor.transpose(tr[:], vt_sb[:, jb * P:(jb + 1) * P], ident[:])
                nc.vector.tensor_copy(
                    v_sb[:, jb, :].rearrange("p (h c) -> p h c", c=128)[:, :, 64:128],
                    tr[:].rearrange("p (h c) -> p h c", c=64))

            # ---- attention ----
            attn_sb = sb.tile([P, SEQ], F32R, tag="at")
            for ig in range(2):
                i0 = ig * 1024
                av_ps = pb.tile([P, HPC, 1024], F32, tag="b")
                for jb in range(16):
                    sts = []
                    for h in range(HPC):
                        st = pa.tile([P, 1024], F32, tag=f"s{h}")
                        for nb in range(2):
                            nc.tensor.matmul(
                                st[:, nb * 512:(nb + 1) * 512],
                                kt_sb[h * DH:(h + 1) * DH, jb * P:(jb + 1) * P],
                                qt_sb[h * DH:(h + 1) * DH,
                                      i0 + nb * 512:i0 + (nb + 1) * 512],
                                start=True, stop=True,
                                tile_position=(h * DH, 0))
                        sts.append(st)
                    for h in range(HPC):
                        e_t = ep.tile([P, 1024], BF16, tag="e")
                        nc.scalar.activation(e_t[:], sts[h][:], AF.Exp,
                                             bias=0.0, scale=1.0)
                        for nb in range(2):
                            nc.tensor.matmul(
                                av_ps[:, h, nb * 512:(nb + 1) * 512],
                                v_sb[:, jb, 128 * h:128 * h + 128],
                                e_t[:, nb * 512:(nb + 1) * 512],
                                start=(jb == 0), stop=(jb == 15))
                # normalize: row 0 of av is sumexp
                for h in range(HPC):
                    se = sb.tile([1, 1024], F32, tag="se")
                    nc.vector.tensor_copy(se[:], av_ps[0:1, h, :])
                    nc.vector.reciprocal(se[:], se[:])
                    rb = sb.tile([P, 1024], F32, tag="rb")
                    nc.gpsimd.partition_broadcast(rb[:], se[:], channels=P)
                    nc.vector.tensor_tensor(
                        attn_sb[h * DH:(h + 1) * DH, i0:i0 + 1024],
                        av_ps[64:128, h, :], rb[64:128, :], ALU.mult)

            # ---- out-proj partial: yp = wo.T @ attn ----
            for m in range(4):
                yp_sb = sb.tile([P, SEQ], F32, tag=f"x{m}")
                for nb in range(4):
                    slot = pa.tile([P, 512], F32, tag=f"s{nb % 2}")
                    nc.tensor.matmul(slot[:], wo_t[:, m * P:(m + 1) * P],
                                     attn_sb[:, nb * 512:(nb + 1) * 512],
                                     start=True, stop=True)
                    nc.vector.tensor_copy(yp_sb[:, nb * 512:(nb + 1) * 512], slot[:])
                nc.sync.dma_start(yp_d[m * P:(m + 1) * P, :], yp_sb[:])

    nc.compile()
    return nc


def kernel(x, Wq, Wk, Wv, Wo, bo, gamma, beta):
    from concourse import bass_utils

    x = np.asarray(x, np.float32)
    Wq, Wk, Wv, Wo = (np.asarray(w, np.float32) for w in (Wq, Wk, Wv, Wo))
    bo, gamma, beta = (np.asarray(v, np.float32) for v in (bo, gamma, beta))
    b = x.shape[0]
    xs = x.reshape(b, C, SEQ)

    s = DH ** -0.5
    aq_f = gamma[:, None] * Wq * s
    ak_f = gamma[:, None] * Wk
    av_f = gamma[:, None] * Wv
    vq_f = (Wq.T @ beta) * s
    vk_f = Wk.T @ beta
    vv_f = Wv.T @ beta

    if "nc" not in _CACHE:
        _CACHE["nc"] = _build()
    nc = _CACHE["nc"]

    in_maps = []
    for core in range(8):
        bi, hg = divmod(core, 4)
        cs = slice(hg * P, (hg + 1) * P)
        in_maps.append({
            "xb": np.ascontiguousarray(xs[bi]),
            "aq": np.ascontiguousarray(aq_f[:, cs]),
            "ak": np.ascontiguousarray(ak_f[:, cs]),
            "av": np.ascontiguousarray(av_f[:, cs]),
            "wo": np.ascontiguousarray(Wo[cs, :]),
            "uq": -aq_f[:, cs].sum(0)[None, :].astype(np.float32),
            "uk": -ak_f[:, cs].sum(0)[None, :].astype(np.float32),
            "uv": -av_f[:, cs].sum(0)[None, :].astype(np.float32),
            "vq": vq_f[cs, None].astype(np.float32),
            "vk": vk_f[cs, None].astype(np.float32),
        })

    global _LAST_IN_MAPS
    _LAST_IN_MAPS = in_maps
    res = bass_utils.run_bass_kernel_spmd(nc, in_maps, core_ids=list(range(8)))
    bias_total = bo + Wo.T @ vv_f
    y = np.empty((b, C, SEQ), np.float32)
    for bi in range(b):
        acc = xs[bi] + bias_total[:, None]
        for hg in range(4):
            acc = acc + res.results[bi * 4 + hg]["yp"]
        y[bi] = acc
    return y.reshape(x.shape).astype(np.float32)



# revision 6
# speedup vs baseline: 1.5904x; 1.5904x over previous
"""Trainium2 Bass kernel for nn_CrossAttentionBlock (LN -> MHA -> out-proj -> residual).

Sharding: 8 cores = 2 batches x 4 head-groups (2 heads each). Each core:
  - LN stats of its batch via ones-matmul (bf16 x); rsqrt via one batched
    Ln+Exp pair (single ACT table set),
  - projects raw x for its 2 heads' Q/K/V (bf16), folding the per-token LN
    scale in at PSUM evacuation and the mean removal as an in-group rank-1,
  - attention: QK bf16, exp on ACT -> fp8 prob slabs, AV as fp8 DoubleRow
    matmuls (2 key blocks per pass) with the [1|0..|V] sumexp ride-along,
  - normalization via reciprocal_approx_fast + gpsimd broadcast,
  - partial out-proj with its Wo rows, bf16 output.
Host sums the 4 partials per batch and adds bias + residual.
Q-projection for the second query half and out-proj of the previous half are
deferred into the attention loop to keep the PE free of >3.4us idle windows
(HAM clock-gate hysteresis: one such window halves the PE clock).
"""
import numpy as np

C = 512
SEQ = 2048
P = 128
NCH = 4          # c chunks of 128
NB = 512         # token column block for stats/proj
DH = 64
HPC = 2          # heads per core
IG = 1024        # i-block (query) width for attention
EPS = 1e-5

_CACHE = {}
_LAST_IN_MAPS = None


def _build():
    import concourse.bass as bass
    import concourse.tile as tile
    from concourse import bacc, mybir
    from concourse.masks import make_identity

    F32 = mybir.dt.float32
    BF16 = mybir.dt.bfloat16
    F8 = mybir.dt.float8e4
    AF = mybir.ActivationFunctionType
    ALU = mybir.AluOpType
    DR = mybir.MatmulPerfMode.DoubleRow

    nc = bacc.Bacc("TRN2", target_bir_lowering=False, debug=False,
                   enable_asserts=False, num_devices=8)

    xb_d = nc.dram_tensor("xb", [C, SEQ], BF16, kind="ExternalInput").ap()
    aq_d = nc.dram_tensor("aq", [C, P], BF16, kind="ExternalInput").ap()
    ak_d = nc.dram_tensor("ak", [C, P], BF16, kind="ExternalInput").ap()
    av_d = nc.dram_tensor("av", [C, P], BF16, kind="ExternalInput").ap()
    wo_d = nc.dram_tensor("wo", [P, C], BF16, kind="ExternalInput").ap()
    uq_d = nc.dram_tensor("uq", [1, P], BF16, kind="ExternalInput").ap()
    uk_d = nc.dram_tensor("uk", [1, P], BF16, kind="ExternalInput").ap()
    uv_d = nc.dram_tensor("uv", [1, P], BF16, kind="ExternalInput").ap()
    vq_d = nc.dram_tensor("vq", [P, 1], F32, kind="ExternalInput").ap()
    yp_d = nc.dram_tensor("yp", [C, SEQ], BF16, kind="ExternalOutput").ap()

    with tile.TileContext(nc) as tc:
        with tc.tile_pool(name="sb", bufs=1) as sb, \
             tc.tile_pool(name="ep", bufs=4) as ep, \
             tc.tile_pool(name="pa", bufs=1, space="PSUM") as pa, \
             tc.tile_pool(name="pb", bufs=1, space="PSUM") as pb:

            # ---- constant / weight loads ----
            aw = {}
            for name, d in (("aq", aq_d), ("ak", ak_d), ("av", av_d)):
                t = sb.tile([P, NCH, P], BF16, tag=name, name=name)
                nc.sync.dma_start(t[:], d.rearrange("(k p) m -> p k m", p=P))
                aw[name] = t
            wo_t = sb.tile([P, C], BF16, tag="wo")
            nc.sync.dma_start(wo_t[:], wo_d[:, :])
            uvec = {}
            for name, d in (("uq", uq_d), ("uk", uk_d), ("uv", uv_d)):
                t = sb.tile([1, P], BF16, tag=name, name=name)
                nc.sync.dma_start(t[:], d[:, :])
                uvec[name] = t
            vq_t = sb.tile([P, 1], F32, tag="vq")
            nc.sync.dma_start(vq_t[:], vq_d[:, :])
            ones_t = sb.tile([P, 2], BF16, tag="ones")
            nc.vector.memset(ones_t[:], 1.0 / C)
            eps_t = sb.tile([1, 1], F32, tag="eps")
            nc.vector.memset(eps_t[:], EPS)
            ident_f = sb.tile([P, P], F32, tag="idf")
            make_identity(nc, ident_f[:])
            ident_b = sb.tile([P, P], BF16, tag="idb")
            nc.vector.tensor_copy(ident_b[:], ident_f[:])

            # ---- x load + square ----
            xt = []
            for k in range(NCH):
                t = sb.tile([P, SEQ], BF16, tag=f"x{k}", name=f"x{k}")
                nc.sync.dma_start(t[:], xb_d[k * P:(k + 1) * P, :])
                xt.append(t)
            xsq = []
            for k in range(NCH):
                t = sb.tile([P, SEQ], BF16, tag=f"q{k}", name=f"xsq{k}")
                nc.vector.tensor_tensor(t[:], xt[k][:], xt[k][:], ALU.mult)
                xsq.append(t)

            # ---- LN stats (per block) + one batched Ln/Exp rsqrt ----
            mu_sb = sb.tile([1, SEQ], F32, tag="mu")
            m_bf = sb.tile([1, SEQ], BF16, tag="mbf")
            musq = sb.tile([1, SEQ], F32, tag="musq")
            varr = sb.tile([1, SEQ], F32, tag="varr")
            lnv = sb.tile([1, SEQ], F32, tag="lnv")
            rs_row = sb.tile([1, SEQ], F32, tag="rsr")
            rs_b = sb.tile([P, SEQ], F32, tag="rsb")
            rs_bf = sb.tile([P, SEQ], BF16, tag="rsbf")
            for nb in range(4):
                sl = slice(nb * NB, (nb + 1) * NB)
                stx = pa.tile([2, NB], F32, tag="s0", name=f"stx{nb}")
                for k in range(NCH):
                    nc.tensor.matmul(stx[:], ones_t[:], xt[k][:, sl],
                                     start=(k == 0), stop=(k == 3))
                sts = pa.tile([2, NB], F32, tag="s1", name=f"sts{nb}")
                for k in range(NCH):
                    nc.tensor.matmul(sts[:], ones_t[:], xsq[k][:, sl],
                                     start=(k == 0), stop=(k == 3))
                nc.vector.tensor_copy(mu_sb[:, sl], stx[0:1, :])
                nc.vector.tensor_copy(m_bf[:, sl], mu_sb[:, sl])
                nc.vector.tensor_tensor(musq[:, sl], mu_sb[:, sl],
                                        mu_sb[:, sl], ALU.mult)
                nc.vector.tensor_tensor(varr[:, sl], sts[0:1, :],
                                        musq[:, sl], ALU.subtract)
            nc.scalar.activation(lnv[:], varr[:], AF.Ln,
                                 bias=eps_t[0:1, :], scale=1.0)
            nc.scalar.activation(rs_row[:], lnv[:], AF.Exp,
                                 bias=0.0, scale=-0.5)
            nc.gpsimd.partition_broadcast(rs_b[:], rs_row[:], channels=P)
            nc.vector.tensor_copy(rs_bf[:], rs_b[:])

            # ---- projections of raw x; LN folded in at evacuation ----
            # q = rs * (Aq^T x + uq (x) mu) + vq   (gamma folded into Aq)
            qt_sb = sb.tile([P, SEQ], BF16, tag="qt")
            kt_sb = sb.tile([P, SEQ], BF16, tag="kt")
            vt_sb = sb.tile([P, SEQ], BF16, tag="vt")
            ptags = ["s0", "s1", "b01", "b23"]
            pn = 0

            def project(wname, uname, dst, bias, nb):
                nonlocal pn
                sl = slice(nb * NB, (nb + 1) * NB)
                pool = pa if ptags[pn % 4] in ("s0", "s1") else pb
                slot = pool.tile([P, NB], F32, tag=ptags[pn % 4],
                                 name=f"pj{nb}{wname}")
                pn += 1
                for k in range(NCH):
                    nc.tensor.matmul(slot[:], aw[wname][:, k, :],
                                     xt[k][:, sl], start=(k == 0), stop=False)
                nc.tensor.matmul(slot[:], uvec[uname][:],
                                 m_bf[:, sl], start=False, stop=True)
                nc.vector.tensor_tensor(dst[:, sl], slot[:],
                                        rs_bf[:, sl], ALU.mult)
                if bias is not None:
                    nc.vector.tensor_scalar(
                        out=dst[:, sl], in0=dst[:, sl],
                        scalar1=bias[:, 0:1], scalar2=None, op0=ALU.add)

            for nb in range(4):
                project("ak", "uk", kt_sb, None, nb)
                project("av", "uv", vt_sb, None, nb)
            project("aq", "uq", qt_sb, vq_t, 0)
            project("aq", "uq", qt_sb, vq_t, 1)

            # ---- V -> fp8 DoubleRow slabs: [1|0*63|V64] per head, 2 j-blocks ----
            v_f8 = sb.tile([P, 8, 2, 256], F8, tag="vsb")
            nc.vector.memset(v_f8[:], 0.0)
            nc.vector.memset(
                v_f8[:].rearrange("p t s (h c) -> p t s h c", c=128)
                [:, :, :, :, 0:1], 1.0)
            for jb in range(16):
                tag = "s0" if jb % 2 == 0 else "s1"
                tr = pa.tile([P, P], BF16, tag=tag, name=f"tr{jb}")
                nc.tensor.transpose(tr[:], vt_sb[:, jb * P:(jb + 1) * P],
                                    ident_b[:])
                nc.vector.tensor_copy(
                    v_f8[:, jb // 2, jb % 2, :]
                    .rearrange("p (h c) -> p h c", c=128)[:, :, 64:128],
                    tr[:].rearrange("p (h c) -> p h c", c=64))

            # ---- attention ----
            attn_sb = sb.tile([P, SEQ], BF16, tag="at")
            yp_sb = [sb.tile([P, SEQ], BF16, tag=f"yp{m}", name=f"yp{m}")
                     for m in range(4)]
            opn = 0
            av_ps = [None, None]

            def attention(ig):
                i0 = ig * IG
                for t in range(8):
                    ept = [None, None]
                    for sub in range(2):
                        jb = 2 * t + sub
                        sts = [None, None]
                        for h in range(HPC):
                            st = pa.tile([P, IG], F32, tag=f"s{h}",
                                         name=f"st{ig}_{jb}_{h}")
                            hsl = slice(h * DH, (h + 1) * DH)
                            for nb in range(2):
                                nc.tensor.matmul(
                                    st[:, nb * 512:(nb + 1) * 512],
                                    kt_sb[hsl, jb * P:(jb + 1) * P],
                                    qt_sb[hsl,
                                          i0 + nb * 512:i0 + (nb + 1) * 512],
                                    start=True, stop=True,
                                    tile_position=(h * DH, 0))
                            sts[h] = st
                        if jb == 0 and ig == 0:
                            # deferred work: q-proj of the second query half
                            # fills the PE while the first exps run
                            project("aq", "uq", qt_sb, vq_t, 2)
                            project("aq", "uq", qt_sb, vq_t, 3)
                            av_ps[0] = pb.tile([P, IG], F32, tag="b01",
                                               name=f"av0g{ig}")
                            av_ps[1] = pb.tile([P, IG], F32, tag="b23",
                                               name=f"av1g{ig}")
                        for h in range(HPC):
                            if sub == 0:
                                ept[h] = ep.tile([P, 2, IG], F8, tag="e",
                                                 name=f"e{ig}_{t}_{h}")
                            nc.scalar.activation(ept[h][:, sub, :], sts[h][:],
                                                 AF.Exp, bias=0.0, scale=1.0)
                    for h in range(HPC):
                        for nb in range(2):
                            nc.tensor.matmul(
                                av_ps[h][:, nb * 512:(nb + 1) * 512],
                                v_f8[:, t, :, 128 * h:128 * h + 128],
                                ept[h][:, :, nb * 512:(nb + 1) * 512],
                                start=(t == 0), stop=(t == 7),
                                perf_mode=DR)

            def normalize(ig):
                i0 = ig * IG
                for h in range(HPC):
                    rec = sb.tile([1, IG], F32, tag=f"rc{h}", name=f"rc{ig}{h}")
                    nc.vector.reciprocal_approx_fast(rec[:], av_ps[h][0:1, :])
                    rb = sb.tile([P, IG], F32, tag=f"rb{h}", name=f"rb{ig}{h}")
                    nc.gpsimd.partition_broadcast(rb[:], rec[:], channels=P)
                    nc.vector.tensor_tensor(
                        attn_sb[h * DH:(h + 1) * DH, i0:i0 + IG],
                        av_ps[h][64:128, :], rb[64:128, :], ALU.mult)

            def outproj(ig):
                nonlocal opn
                i0 = ig * IG
                for m in range(4):
                    tag = "b01" if opn % 2 == 0 else "b23"
                    opn += 1
                    slot = pb.tile([P, IG], F32, tag=tag, name=f"op{ig}{m}")
                    for nb in range(2):
                        nc.tensor.matmul(
                            slot[:, nb * 512:(nb + 1) * 512],
                            wo_t[:, m * P:(m + 1) * P],
                            attn_sb[:, i0 + nb * 512:i0 + (nb + 1) * 512],
                            start=True, stop=True)
                    nc.vector.tensor_copy(yp_sb[m][:, i0:i0 + IG], slot[:])

            attention(0)
            normalize(0)
            av_ps[0] = pb.tile([P, IG], F32, tag="b01", name="av0g1")
            av_ps[1] = pb.tile([P, IG], F32, tag="b23", name="av1g1")
            attention(1)
            outproj(0)
            normalize(1)
            outproj(1)
            for m in range(4):
                nc.sync.dma_start(yp_d[m * P:(m + 1) * P, :], yp_sb[m][:])

    nc.compile()
    return nc


def kernel(x, Wq, Wk, Wv, Wo, bo, gamma, beta):
    import ml_dtypes
    from concourse import bass_utils

    BF = ml_dtypes.bfloat16
    x = np.asarray(x, np.float32)
    Wq, Wk, Wv, Wo = (np.asarray(w, np.float32) for w in (Wq, Wk, Wv, Wo))
    bo, gamma, beta = (np.asarray(v, np.float32) for v in (bo, gamma, beta))
    b = x.shape[0]
    xs = x.reshape(b, C, SEQ)
    xs_bf = xs.astype(BF)

    s = DH ** -0.5
    aq_f = gamma[:, None] * Wq * s
    ak_f = gamma[:, None] * Wk
    av_f = gamma[:, None] * Wv
    vq_f = (Wq.T @ beta) * s
    vv_f = Wv.T @ beta

    if "nc" not in _CACHE:
        _CACHE["nc"] = _build()
    nc = _CACHE["nc"]

    in_maps = []
    for core in range(8):
        bi, hg = divmod(core, 4)
        cs = slice(hg * P, (hg + 1) * P)
        in_maps.append({
            "xb": np.ascontiguousarray(xs_bf[bi]),
            "aq": np.ascontiguousarray(aq_f[:, cs].astype(BF)),
            "ak": np.ascontiguousarray(ak_f[:, cs].astype(BF)),
            "av": np.ascontiguousarray(av_f[:, cs].astype(BF)),
            "wo": np.ascontiguousarray(Wo[cs, :].astype(BF)),
            "uq": -aq_f[:, cs].sum(0)[None, :].astype(BF),
            "uk": -ak_f[:, cs].sum(0)[None, :].astype(BF),
            "uv": -av_f[:, cs].sum(0)[None, :].astype(BF),
            "vq": vq_f[cs, None].astype(np.float32),
        })

    global _LAST_IN_MAPS
    _LAST_IN_MAPS = in_maps
    res = bass_utils.run_bass_kernel_spmd(nc, in_maps, core_ids=list(range(8)))
    bias_total = bo + Wo.T @ vv_f
    y = np.empty((b, C, SEQ), np.float32)
    for bi in range(b):
        acc = xs[bi] + bias_total[:, None]
        for hg in range(4):
            acc = acc + res.results[bi * 4 + hg]["yp"].astype(np.float32)
        y[bi] = acc
    return y.reshape(x.shape).astype(np.float32)
